# revision 48
# baseline (speedup 1.0000x reference)
"""Trainium2 Bass kernel for nn_Attention_15109694948028.

Single-layer attention block: QKV proj -> 8-head SDPA (S=4096, d_k=64)
-> out proj -> residual -> LayerNorm.  fp32 I/O.

Sharding: sequence-parallel across 8 NeuronCores.  Core i computes the
full output for query rows [i*512, (i+1)*512) of all 8 heads; K and V
are computed redundantly on every core, so there are NO collectives --
cores are fully independent and the host just concatenates the 8
output slices.  (remote_dma and DRAM collectives are both
non-functional under the axon client runtime, so comm-based K/V
de-duplication is not available.)

On-core dataflow (Q/K/V projections in fp8e4m3 with DoubleRow perf
mode; scores/ctx in bf16; out-proj/transposes in float32r;
residual/LayerNorm in fp32; final rel err ~2.8e-3, budget 2e-2):

  phase A: Q projection via fp8 DoubleRow matmuls (contraction 512 = 2
           DR matmuls of 2x128 packed rows -> 2x fewer PE columns than
           bf16; host packs W as [h*128+i, g*512+c] = W[h*256+g*128+i,c]
           and x^T chunks as [128, 2*N] column-block pairs).  The PSUM
           result is split into 8 per-HEAD zero-padded qt tiles
           (head h's 64 dims at their native partitions, other 64
           partitions memset 0) so scores run full 128-contraction.
  flat attention stream over (round, pair, ktile) steps, k in four
  1024-row rounds (bounds SBUF):
    scores^T st tiles [128 k, 1024 = 2 heads x 512 q] on PE, both
    matmuls share the SAME un-split K pair-tile stationary (the zero
    half of each qt kills the other head's contribution); exp on
    ScalarE (scale=1/8 folds 1/sqrt(d_k); no max-subtract: scores are
    ~N(0,1) so exp cannot overflow).
    FLIPPED ctx matmuls: stationary = es [128 k, 128 q] slice, moving
    = [V_h | ones] [128 k, 65] -> out ctx [128 q, 65] uses all 128
    output partitions; the ones column accumulates the softmax
    denominators.  Per pair the 2x4 qtile regions live in two 1-bank
    PSUM tiles; ONLY the first region matmul of a round carries
    start=True because start resets the whole PSUM bank.
    Emission is software-pipelined ACROSS pair and round boundaries
    with _LOOKAHEAD=2 (scores for step i+2 before ctx for step i) so
    neither ACT nor the in-order PE stalls on the exp round-trip, and
    round r+1's K/V fp8-DR projection matmuls are interleaved as
    filler EMITTED FIRST in each step (no dep on exp -> PE chews them
    while waiting, keeping the HAM clock gate warm).
    Cross-round ctx accumulation lives in SBUF cacc [128 q, 4x65]
    per head (DVE add per round).
  normalize: per (head, qtile) reciprocal of the denominator column
           (DVE) and tensor_scalar_mul into per-pair ctxN tiles laid
           out [128 q, qt x (2 heads' 64 d)] so ONE PE transpose per
           qtile (via f32r identity matmul) yields ctxT pairs
           [128 = 2 heads' d, 512 q].
  phase D: out proj as 4 pair-matmuls per q-subtile (contraction 128)
           + residual (exact fp32 x_slice) + LayerNorm on DVE/ACT --
           INTERLEAVED with the last pair's transposes (q-subtile qs
           starts right after that pair's qt=qs transpose, borrowing
           the idle psB PSUM slots) so the LN chains overlap the
           remaining transposes instead of all queuing at the end.

Benchmarking support: _build_nc(bench_reps=N) wraps the whole body in
an on-device For_i loop (used by test.py to cancel the ~200 ms axon
per-call dispatch latency); the graded kernel() path is loop-free.

Perf history (HW exec, per-rep steady state): baseline bf16 236-270us
-> fp8-DR projections 204us -> padded-128 scores 202us (no change, but
enables shared-stationary scores) -> 2-step lookahead 186us -> filler
first 184us.  Measured-but-rejected: es/vpq in fp8 (196us), manual
tile rings (196us), es bufs 2/3/5/6 (196-258us), SBUF pad shims
(190-222us), LA=3/4 (186-189us).  Known walls: exp stream paces at
~1.4us/step (back-to-back ACT can do ~0.65us but that regime did not
transfer into the kernel; es-buffer-count/placement effects on ACT
write throughput are large, nonlinear, and layout-lottery-like); the
PE-dense floor without exp (ablate noact) is ~176us.
"""

import numpy as np

import concourse.bacc as bacc
import concourse.tile as tile
from concourse import mybir
from concourse.bass_utils import run_bass_kernel_spmd

f32 = mybir.dt.float32
f32r = mybir.dt.float32r
AF = mybir.ActivationFunctionType

S = 4096
D = 512
H = 8
DK = 64
NCORES = 8
SLICE = S // NCORES          # 512 query rows per core
P = 128                      # partitions
HALF = S // 2                # 2048 k rows per half
NKTH = HALF // P             # 16 k-tiles per half
NCHH = HALF // 512           # 4 x^T chunks per half
NQS = SLICE // P             # 4 q-subtiles
KGRP = 2                     # k-tiles per exp batch (2 PSUM banks)
EPS = 1e-5


def _round_f32r(a: np.ndarray) -> np.ndarray:
    """RNE to 11 explicit mantissa bits (matches HW fp32 -> f32r cast)."""
    b = np.ascontiguousarray(a, dtype=np.float32).view(np.uint32)
    lsb = (b >> np.uint32(12)) & np.uint32(1)
    return ((b + np.uint32(0x7FF) + lsb) & np.uint32(0xFFFFF000)).view(np.float32)


import os as _os
_ABLATE = _os.environ.get("KABL", "full")
# attention-path matmul dtype: "f32r" (TF32-class, ~1.6e-5 err),
# "mixed" (f32r projections, bf16 scores/ctx), "bf16" (all bf16, fastest)
_ATTN_DT = "bf16"
_ES_BUFS = int(_os.environ.get("KESBUFS", "4"))
# software-pipeline depth: scores for step i+LOOKAHEAD are emitted before
# ctx for step i, so ACT always has the next exp's input ready and runs
# its fast back-to-back path (~641ns/instr vs ~1450ns isolated)
_LOOKAHEAD = int(_os.environ.get("KLA", "2"))
# es/vpq (ctx operands) in fp8e4m3: halves ACT's es write traffic and the
# PE's ctx stationary reads; exp folds a -2 bias so es stays under fp8 max
# (cancels in the softmax normalization)
_ES8 = _os.environ.get("KES8", "0") == "1"
# SBUF placement shim: a dummy pad tile allocated first shifts every
# subsequent SBUF allocation; engine-vs-engine SBUF contention is strongly
# placement-dependent (measured +-35us swings), so this is a tunable
_KPAD = int(_os.environ.get("KPAD", "0"))
# where next-round K/V projection matmuls are emitted relative to the
# attention stream: "inner" (between ktiles), "pair" (bursts at pair
# boundaries), "round" (all between rounds)
_FILL_MODE = "inner"
_DEBUG_OUT = False
bf16 = mybir.dt.bfloat16
fp8 = mybir.dt.float8e4
DRMODE = mybir.MatmulPerfMode.DoubleRow


def _build_nc(has_bias: bool, has_bo: bool, has_gamma: bool, has_beta: bool,
              bench_reps: int = 0):
    ablate = _ABLATE
    attn_dt = _ATTN_DT
    fill_mode = _FILL_MODE
    # Projections run in fp8e4m3 with DoubleRow perf mode: contraction 512
    # = 2 DR matmuls of 2x128 packed rows, 2x fewer PE columns than bf16.
    proj_dt = fp8
    # dtype of scores matmul operands (qt/ktq tiles)
    sc_dt = f32r if attn_dt == "f32r" else bf16
    # dtype of ctx matmul operands (es/vpq tiles)
    es_dt = fp8 if _ES8 else sc_dt
    nc = bacc.Bacc("TRN2", target_bir_lowering=False, debug=False)

    xT = nc.dram_tensor("xT", [D + 1, S], proj_dt, kind="ExternalInput")
    # DR-packed weights: [h*128+i, g*512+c] = W[h*256+g*128+i, c]
    wq = nc.dram_tensor("wq", [2 * P, 2 * D], proj_dt, kind="ExternalInput")
    wk = nc.dram_tensor("wk", [2 * P, 2 * D], proj_dt, kind="ExternalInput")
    wv = nc.dram_tensor("wv", [2 * P, 2 * D], proj_dt, kind="ExternalInput")
    if has_bias:
        wqb_d = nc.dram_tensor("wqb", [1, D], proj_dt, kind="ExternalInput")
        wkb_d = nc.dram_tensor("wkb", [1, D], proj_dt, kind="ExternalInput")
        wvb_d = nc.dram_tensor("wvb", [1, D], proj_dt, kind="ExternalInput")
    wo = nc.dram_tensor("wo", [D, D], f32r, kind="ExternalInput")
    ident = nc.dram_tensor("ident", [P, P], f32r, kind="ExternalInput")
    xq = nc.dram_tensor("xq", [D + 1, SLICE], proj_dt, kind="ExternalInput")
    xs = nc.dram_tensor("x_slice", [SLICE, D], f32, kind="ExternalInput")
    bo = nc.dram_tensor("bo", [1, D], f32, kind="ExternalInput")
    gamma = nc.dram_tensor("gamma", [1, D], f32, kind="ExternalInput")
    beta = nc.dram_tensor("beta", [1, D], f32, kind="ExternalInput")
    y = nc.dram_tensor("y", [SLICE, D], f32, kind="ExternalOutput")
    if _DEBUG_OUT:
        dbg_cacc = nc.dram_tensor("dbg_cacc", [2 * P, 4 * 65], f32,
                                  kind="ExternalOutput")
        dbg_ctxT = nc.dram_tensor("dbg_ctxT", [P, SLICE], f32,
                                  kind="ExternalOutput")

    with tile.TileContext(nc) as tc:
        def emit_body():
            with (
                tc.tile_pool(name="qt", bufs=1) as qtp,
                tc.tile_pool(name="cacc", bufs=1) as cap,
                tc.tile_pool(name="consts", bufs=1) as cp,
            ):
                if _KPAD:
                    padt = cp.tile([P, _KPAD], f32, tag="pad")
                    nc.gpsimd.memset(padt[:], 0.0)
                eps_t = cp.tile([P, 1], f32, tag="eps")
                nc.gpsimd.memset(eps_t[:], EPS)
                eb_t = None
                if _ES8:
                    eb_t = cp.tile([P, 1], f32, tag="eb")
                    nc.gpsimd.memset(eb_t[:], -2.0)

                def bcast_row(dram_row, tag):
                    r = cp.tile([1, D], f32, tag=f"{tag}_row", name=f"{tag}_row")
                    nc.sync.dma_start(r[:], dram_row)
                    b = cp.tile([P, D], f32, tag=f"{tag}_b", name=f"{tag}_b")
                    nc.gpsimd.partition_broadcast(b[:], r[0:1, :])
                    return b

                bo_b = bcast_row(bo[:], "bo") if has_bo else None
                gamma_b = bcast_row(gamma[:], "gamma") if has_gamma else None
                beta_b = bcast_row(beta[:], "beta") if has_beta else None
                if has_bias:
                    xt_ones = cp.tile([1, S], proj_dt, tag="xt_ones")
                    nc.sync.dma_start(xt_ones[:], xT[D:D + 1, :])

                # per-HEAD query tiles, zero-padded to full 128-partition
                # contraction: qtS[2p] holds h0's Q rows at partitions 0:64
                # (zeros at 64:128), qtS[2p+1] h1's at 64:128 (zeros at
                # 0:64).  Scores then run contraction-128 matmuls against
                # the un-split K pair tile, which streams measurably faster
                # per column than the 64-partition variant.
                qtS = [qtp.tile([P, SLICE], sc_dt, tag=f"qt{hh}", name=f"qt{hh}")
                       for hh in range(H)]
                for pp in range(4):
                    nc.gpsimd.memset(qtS[2 * pp][DK:P, :], 0.0)
                    nc.gpsimd.memset(qtS[2 * pp + 1][0:DK, :], 0.0)
                # flipped-ctx accumulator: [128 q, 4 qtiles x (64 d + denom)]
                caccS = [cap.tile([P, 4 * 65], f32, tag=f"cacc{h}",
                                  name=f"cacc{h}") for h in range(H)]
                ident_t = cp.tile([P, P], f32r, tag="ident")

                # ---- phase A: Q projection from per-core xq ----
                with (
                    tc.tile_pool(name="wqp", bufs=1) as wqp,
                    tc.tile_pool(name="xqp", bufs=1) as xqp,
                    tc.tile_pool(name="psA", bufs=4, space="PSUM") as psA,
                ):
                    wqS = [wqp.tile([P, 2 * D], proj_dt, tag=f"wq{h}",
                                    name=f"wq{h}") for h in range(2)]
                    for h in range(2):
                        nc.sync.dma_start(wqS[h][:], wq[h * P:(h + 1) * P, :])
                    xqS = [xqp.tile([P, 2 * SLICE], proj_dt, tag=f"xq{h}",
                                    name=f"xq{h}") for h in range(2)]
                    for h in range(2):
                        for g in range(2):
                            r0 = 256 * h + 128 * g
                            nc.sync.dma_start(
                                xqS[h][:, g * SLICE:(g + 1) * SLICE],
                                xq[r0:r0 + P, :])
                    if has_bias:
                        wqb = wqp.tile([1, D], proj_dt, tag="wqb")
                        nc.sync.dma_start(wqb[:], wqb_d[:])
                        xq_ones = wqp.tile([1, SLICE], proj_dt, tag="xq_ones")
                        nc.sync.dma_start(xq_ones[:], xq[D:D + 1, :])
                    wqv = [t.rearrange("p (g c) -> p g c", g=2) for t in wqS]
                    xqv = [t.rearrange("p (g c) -> p g c", g=2) for t in xqS]
                    NQ4 = SLICE // 4
                    for p in range(4):
                        ps = psA.tile([P, SLICE], f32, tag="psA")
                        idx = 0
                        for n in range(2):
                            for h in range(2):
                                nc.tensor.matmul(
                                    ps[:, n * NQ4 * 2:(n + 1) * NQ4 * 2],
                                    wqv[h][:, :, p * P:(p + 1) * P],
                                    xqv[h][:, :, n * NQ4 * 2:(n + 1) * NQ4 * 2],
                                    start=(idx == 0),
                                    stop=(idx == 3 and not has_bias),
                                    perf_mode=DRMODE, skip_group_check=True,
                                )
                                idx += 1
                        if has_bias:
                            nc.tensor.matmul(
                                ps[:], wqb[0:1, p * P:(p + 1) * P], xq_ones[:],
                                start=False, stop=True,
                            )
                        nc.vector.tensor_copy(qtS[2 * p][0:DK, :], ps[0:DK, :])
                        nc.vector.tensor_copy(qtS[2 * p + 1][DK:P, :],
                                              ps[DK:P, :])

                # ---- round loop: K/V proj per k-quarter + attention ----
                # Round r+1's K/V projection work is EMITTED between round
                # r's attention pairs: engines execute their streams in
                # emission order, so this is what actually interleaves the
                # projection matmuls into the ACT(exp)-paced attention.
                QK = S // 4          # 1024 k rows per round
                NKTQ = QK // P       # 8 k-tiles per round
                with (
                    tc.tile_pool(name="wkv", bufs=1) as wkvp,
                    tc.tile_pool(name="ktq", bufs=2) as ktqp,
                    tc.tile_pool(name="vpq", bufs=2) as vpqp,
                    tc.tile_pool(name="xtc", bufs=2) as xtcp,
                    tc.tile_pool(name="es", bufs=_ES_BUFS) as esp,
                    tc.tile_pool(name="nrm", bufs=1) as nrmp,
                    tc.tile_pool(name="ctxT", bufs=1) as ctp,
                ):
                    wkS = [wkvp.tile([P, 2 * D], proj_dt, tag=f"wk{h}",
                                     name=f"wk{h}") for h in range(2)]
                    wvS = [wkvp.tile([P, 2 * D], proj_dt, tag=f"wv{h}",
                                     name=f"wv{h}") for h in range(2)]
                    for h in range(2):
                        nc.sync.dma_start(wkS[h][:], wk[h * P:(h + 1) * P, :])
                    wkv = [t.rearrange("p (g c) -> p g c", g=2) for t in wkS]
                    wvv = [t.rearrange("p (g c) -> p g c", g=2) for t in wvS]
                    if has_bias:
                        wkb = wkvp.tile([1, D], proj_dt, tag="wkb")
                        wvb = wkvp.tile([1, D], proj_dt, tag="wvb")
                        nc.sync.dma_start(wkb[:], wkb_d[:])
                        nc.sync.dma_start(wvb[:], wvb_d[:])
                    # per-pair normalized ctx [128 q, 4 qtiles x (2 heads' d)]
                    # and per-pair transposed ctx [128 = 2 heads' d, 512 q]
                    ctxN = [ctp.tile([P, 4 * 2 * DK], f32r, tag=f"ctxN{pp}",
                                     name=f"ctxN{pp}") for pp in range(4)]
                    ctxTp = [ctp.tile([P, SLICE], f32r, tag=f"ctxTp{pp}",
                                      name=f"ctxTp{pp}") for pp in range(4)]
                    woS = [wkvp.tile([P, D], f32r, tag=f"wo{pp}",
                                     name=f"wo{pp}") for pp in range(4)]

                    with (
                        tc.tile_pool(name="st", bufs=2, space="PSUM") as stp,
                        tc.tile_pool(name="psB", bufs=2, space="PSUM") as psBp,
                        tc.tile_pool(name="ctxps", bufs=2, space="PSUM") as cpp,
                        tc.tile_pool(name="ln", bufs=2) as lnp,
                    ):
                        ktq_of, vpq_of, xc_of = {}, {}, {}

                        def alloc_round(r):
                            ktq_of[r] = [ktqp.tile([P, QK], sc_dt, tag=f"ktq{p}",
                                                   name=f"ktq{p}") for p in range(4)]
                            vpq_of[r] = [vpqp.tile([P, H * 65], es_dt, tag=f"vpq{t}",
                                                   name=f"vpq{t}") for t in range(NKTQ)]
                            xc_of[r] = [xtcp.tile([P, 2 * QK], proj_dt, tag=f"xtc{h}",
                                                  name=f"xtc{h}") for h in range(2)]
                            k0 = r * QK
                            for h in range(2):
                                for g in range(2):
                                    r0 = 256 * h + 128 * g
                                    nc.sync.dma_start(
                                        xc_of[r][h][:, g * QK:(g + 1) * QK],
                                        xT[r0:r0 + P, k0:k0 + QK])

                        def kv_fill_gen(r):
                            """Yield after each PE op of round r's K/V
                            projection -- consumed one step at a time as
                            filler inside the previous round's attention."""
                            if ablate == "noproj":
                                for p in range(4):
                                    kv = ktq_of[r][p][:]
                                    nc.gpsimd.memset(kv.bitcast(f32) if sc_dt == f32r else kv, 0.5)
                                for t in range(NKTQ):
                                    vv = vpq_of[r][t][:]
                                    nc.gpsimd.memset(vv.bitcast(f32) if es_dt == f32r else vv, 0.5)
                                return
                            k0 = r * QK
                            xc, ktq, vpq = xc_of[r], ktq_of[r], vpq_of[r]
                            xcv = [t.rearrange("p (g c) -> p g c", g=2)
                                   for t in xc]
                            for cc in range(QK // 512):
                                for p in range(4):
                                    ps = psBp.tile([P, 512], f32, tag="psB")
                                    idx = 0
                                    for n in range(2):
                                        for h in range(2):
                                            nc.tensor.matmul(
                                                ps[:, n * 256:(n + 1) * 256],
                                                wkv[h][:, :, p * P:(p + 1) * P],
                                                xcv[h][:, :, cc * 512 + n * 256:
                                                       cc * 512 + (n + 1) * 256],
                                                start=(idx == 0),
                                                stop=(idx == 3 and not has_bias),
                                                perf_mode=DRMODE,
                                                skip_group_check=True,
                                            )
                                            idx += 1
                                            if idx == 2:
                                                yield
                                    if has_bias:
                                        nc.tensor.matmul(
                                            ps[:], wkb[0:1, p * P:(p + 1) * P],
                                            xt_ones[0:1, k0 + cc * 512:k0 + (cc + 1) * 512],
                                            start=False, stop=True,
                                        )
                                    nc.vector.tensor_copy(
                                        ktq[p][:, cc * 512:(cc + 1) * 512], ps[:])
                                    yield
                            for lt in range(NKTQ):
                                ps = psBp.tile([P, D], f32, tag="psB")
                                idx = 0
                                for n in range(2):
                                    for h in range(2):
                                        nc.tensor.matmul(
                                            ps[:, n * 256:(n + 1) * 256],
                                            xcv[h][:, :, lt * P:(lt + 1) * P],
                                            wvv[h][:, :, n * 256:(n + 1) * 256],
                                            start=(idx == 0),
                                            stop=(idx == 3 and not has_bias),
                                            perf_mode=DRMODE,
                                            skip_group_check=True,
                                        )
                                        idx += 1
                                        if idx == 2:
                                            yield
                                if has_bias:
                                    nc.tensor.matmul(
                                        ps[:],
                                        xt_ones[0:1, k0 + lt * P:k0 + (lt + 1) * P],
                                        wvb[:], start=False, stop=True,
                                    )
                                vt = vpq[lt]
                                v3 = vt.rearrange("p (h c) -> p h c", c=65)
                                # ones columns on DVE (not Pool): the ctx
                                # matmul already waits on the DVE V-copy, so
                                # this adds no cross-engine hop
                                if es_dt == f32r:
                                    nc.vector.memset(v3[:, :, 64:65].bitcast(f32), 1.0)
                                else:
                                    nc.vector.memset(v3[:, :, 64:65], 1.0)
                                nc.vector.tensor_copy(
                                    v3[:, :, 0:64],
                                    ps[:].rearrange("p (h d) -> p h d", d=DK))
                                yield

                        def normalize(h):
                            # denom col qt*65+64 -> rc[:, qt]; ctxN = cacc * rc
                            # head h's d-block sits at cols qt*128 + (h%2)*64
                            # of the pair tile so one transpose per qtile
                            # yields both heads stacked on partitions
                            rc = nrmp.tile([P, 4], f32, tag="rc", bufs=2)
                            cN = ctxN[h // 2]
                            off = (h % 2) * DK
                            for qt in range(4):
                                nc.vector.reciprocal(
                                    rc[:, qt:qt + 1],
                                    caccS[h][:, qt * 65 + 64:qt * 65 + 65])
                            for qt in range(4):
                                nc.vector.tensor_scalar_mul(
                                    cN[:, qt * P + off:qt * P + off + DK],
                                    caccS[h][:, qt * 65:qt * 65 + DK],
                                    rc[:, qt:qt + 1])

                        def emit_phase_d(qs):
                            # out proj + residual + LayerNorm for q-subtile
                            # qs, PSUM slot borrowed from the idle psB pool
                            op = psBp.tile([P, D], f32, tag="psB",
                                           name=f"psD{qs}")
                            for pp in range(4):
                                nc.tensor.matmul(
                                    op[:], ctxTp[pp][:, qs * P:(qs + 1) * P],
                                    woS[pp][:],
                                    start=(pp == 0), stop=(pp == 3),
                                )
                            xt_ = lnp.tile([P, D], f32, tag="xres")
                            nc.sync.dma_start(xt_[:], xs[qs * P:(qs + 1) * P, :])
                            t = lnp.tile([P, D], f32, tag="t")
                            nc.vector.tensor_add(t[:], op[:], xt_[:])
                            if has_bo:
                                nc.vector.tensor_add(t[:], t[:], bo_b[:])
                            s1 = lnp.tile([P, 1], f32, tag="s1")
                            nc.vector.reduce_sum(s1[:], t[:],
                                                 axis=mybir.AxisListType.X)
                            negmu = lnp.tile([P, 1], f32, tag="negmu")
                            nc.vector.tensor_scalar_mul(negmu[:], s1[:], -1.0 / D)
                            tcen = lnp.tile([P, D], f32, tag="tcen")
                            nc.vector.tensor_scalar_add(tcen[:], t[:], negmu[:])
                            sq = lnp.tile([P, D], f32, tag="sq")
                            v1 = lnp.tile([P, 1], f32, tag="v1")
                            nc.scalar.activation(sq[:], tcen[:], AF.Square,
                                                 accum_out=v1[:])
                            std = lnp.tile([P, 1], f32, tag="std")
                            nc.scalar.activation(std[:], v1[:], AF.Sqrt,
                                                 bias=eps_t[:], scale=1.0 / D)
                            rstd = lnp.tile([P, 1], f32, tag="rstd")
                            nc.vector.reciprocal(rstd[:], std[:])
                            out_t = lnp.tile([P, D], f32, tag="out_t")
                            nc.vector.tensor_scalar_mul(out_t[:], tcen[:], rstd[:])
                            if has_gamma:
                                nc.vector.tensor_mul(out_t[:], out_t[:], gamma_b[:])
                            if has_beta:
                                nc.vector.tensor_add(out_t[:], out_t[:], beta_b[:])
                            nc.sync.dma_start(y[qs * P:(qs + 1) * P, :], out_t[:])

                        def transpose_pair(p):
                            # ctxN [128 q, 128 = 2 heads' d] blocks ->
                            # ctxTp [128 d, 512 q] via one PE transpose each.
                            # For the LAST pair, out-proj + LN for q-subtile
                            # qt follows immediately after qt's transpose so
                            # the phase-D chains overlap the remaining
                            # transposes instead of all waiting at the end.
                            if ablate == "notrans":
                                nc.vector.tensor_copy(ctxTp[p][:], qtS[p][:])
                                if p == 3:
                                    for qs in range(NQS):
                                        emit_phase_d(qs)
                                return
                            for qt in range(4):
                                psT = psBp.tile([P, P], f32r, tag="psB")
                                nc.tensor.matmul(
                                    psT[:],
                                    ctxN[p][:, qt * P:(qt + 1) * P],
                                    ident_t[:], is_transpose=True,
                                )
                                nc.vector.tensor_copy(
                                    ctxTp[p][:, qt * P:(qt + 1) * P], psT[:])
                                if p == 3:
                                    emit_phase_d(qt)

                        esq = []
                        ctx_of = {}

                        es_const = None
                        if ablate in ("desync", "noact"):
                            es_const = ctp.tile([P, 1024], es_dt,
                                                tag="es_const")
                            nc.gpsimd.memset(es_const[:], 0.5)
                        sttz = None
                        if ablate == "noscore":
                            sttz = stp.tile([P, 1024], f32, tag="st")
                            nc.vector.memset(sttz[:], 0.25)

                        def emit_s(rnd, p, lkt):
                            ktq = ktq_of[rnd]
                            si = (rnd * 4 + p) * NKTQ + lkt
                            if ablate != "noscore":
                                stt = stp.tile([P, 1024], f32, tag="st")
                                nc.tensor.matmul(
                                    stt[:, 0:512],
                                    ktq[p][:, lkt * P:(lkt + 1) * P],
                                    qtS[2 * p][:],
                                    start=True, stop=True,
                                )
                                nc.tensor.matmul(
                                    stt[:, 512:1024],
                                    ktq[p][:, lkt * P:(lkt + 1) * P],
                                    qtS[2 * p + 1][:],
                                    start=True, stop=True,
                                )
                            if ablate == "noact":
                                esq.append(es_const)
                                return
                            es = esp.tile([P, 1024], es_dt, tag="es")
                            kw = dict(bias=eb_t[:]) if _ES8 else {}
                            if ablate == "dveexp":
                                nc.vector.tensor_copy(es[:], stt[:])
                            elif ablate == "noscore":
                                nc.scalar.activation(es[:], sttz[:], AF.Exp,
                                                     scale=0.125, **kw)
                            else:
                                nc.scalar.activation(es[:], stt[:], AF.Exp,
                                                     scale=0.125, **kw)
                            esq.append(es_const if ablate == "desync" else es)

                        def emit_c(rnd, p, lkt):
                            # flipped ctx: stationary = es [128 k, 128 q],
                            # moving = [V_h | ones] [128 k, 65]; out
                            # [128 q, 65] regions; alternate the two
                            # PSUM banks (ctx0/ctx1) between matmuls
                            vpq = vpq_of[rnd]
                            h0, h1 = 2 * p, 2 * p + 1
                            es = esq.pop(0)
                            if lkt == 0:
                                cA = cpp.tile([P, 4 * 65], f32, tag="ctxu",
                                              name=f"ctxu{rnd}_{p}a")
                                cB = cpp.tile([P, 4 * 65], f32, tag="ctxu",
                                              name=f"ctxu{rnd}_{p}b")
                                ctx_of[p] = (cA, cB)
                            ctx0, ctx1 = ctx_of[p]
                            if ablate == "noctx":
                                if lkt == 0:
                                    nc.tensor.matmul(
                                        ctx0[:, 0:260], es[:, 0:128],
                                        vpq[lkt][:, 0:260],
                                        start=True, stop=True)
                                    nc.tensor.matmul(
                                        ctx1[:, 0:260], es[:, 0:128],
                                        vpq[lkt][:, 0:260],
                                        start=True, stop=True)
                                return
                            for qt in range(4):
                                for hb, ctx, hh in ((0, ctx0, h0),
                                                    (1, ctx1, h1)):
                                    # start=True resets the WHOLE PSUM
                                    # bank, so only the first region
                                    # matmul of the round may carry it
                                    nc.tensor.matmul(
                                        ctx[:, qt * 65:(qt + 1) * 65],
                                        es[:, hb * 512 + qt * P:
                                           hb * 512 + (qt + 1) * P],
                                        vpq[lkt][:, hh * 65:(hh + 1) * 65],
                                        start=(lkt == 0 and qt == 0),
                                        stop=(lkt == NKTQ - 1 and qt == 3),
                                        skip_group_check=True,
                                    )

                        def finish_pair(rnd, p):
                            h0, h1 = 2 * p, 2 * p + 1
                            ctx0, ctx1 = ctx_of[p]
                            for h, ctxu in ((h0, ctx0), (h1, ctx1)):
                                if rnd == 0:
                                    nc.vector.tensor_copy(caccS[h][:], ctxu[:])
                                else:
                                    nc.vector.tensor_add(caccS[h][:], ctxu[:],
                                                         caccS[h][:])
                                if rnd == 3:
                                    normalize(h)
                            if rnd == 3:
                                transpose_pair(p)

                        alloc_round(0)
                        # wv/wo/ident are needed later than round 0's first
                        # K-proj matmuls: issue them after the x^T chunks so
                        # the fill isn't stuck behind cold weights
                        for h in range(2):
                            nc.sync.dma_start(wvS[h][:], wv[h * P:(h + 1) * P, :])
                        for pp in range(4):
                            nc.sync.dma_start(woS[pp][:], wo[pp * P:(pp + 1) * P, :])
                        nc.sync.dma_start(ident_t[:], ident[:])
                        for _ in kv_fill_gen(0):
                            pass
                        if ablate != "noattn":
                            # flat software-pipelined stream over all
                            # (round, pair, ktile) steps: scores for step
                            # i+1 are emitted before ctx for step i, ACROSS
                            # pair and round boundaries, so the in-order PE
                            # never stalls on exp at pair starts
                            steps = [(r, p, k) for r in range(4)
                                     for p in range(4) for k in range(NKTQ)]
                            filler = None
                            la = _LOOKAHEAD
                            for j in range(la):
                                emit_s(*steps[j])
                            alloc_round(1)
                            for i, (rnd, p, lkt) in enumerate(steps):
                                if lkt == 0 and p == 0:
                                    if filler is not None:
                                        for _ in filler:
                                            pass
                                    filler = (kv_fill_gen(rnd + 1)
                                              if rnd < 3 else None)
                                if lkt == 0 and p == 2 and rnd < 2:
                                    # prefetch round rnd+2's x^T chunks a
                                    # full round before its fillers run
                                    alloc_round(rnd + 2)
                                # filler FIRST: the proj matmuls have no dep
                                # on exp, so the in-order PE chews them while
                                # waiting for exp to free the stt buffer /
                                # produce es -- no micro-idles (HAM stays
                                # warm at 2.4GHz)
                                if filler is not None:
                                    next(filler, None)
                                if i + la < len(steps):
                                    emit_s(*steps[i + la])
                                emit_c(rnd, p, lkt)
                                if lkt == NKTQ - 1:
                                    finish_pair(rnd, p)
                            if filler is not None:
                                for _ in filler:
                                    pass
                        else:
                            for rnd in range(4):
                                if rnd < 3:
                                    alloc_round(rnd + 1)
                                    for _ in kv_fill_gen(rnd + 1):
                                        pass
                        if ablate == "noattn":
                            for h in range(H):
                                nc.gpsimd.memset(caccS[h][:], 1.0)
                                normalize(h)
                            for p in range(4):
                                transpose_pair(p)


        if bench_reps:
            with tc.For_i(0, bench_reps, 1):
                emit_body()
        else:
            emit_body()
    nc.compile()
    return nc


_NC_CACHE: dict = {}


def _get_nc(flags, bench_reps: int = 0):
    key = (flags, bench_reps, _ABLATE, _ATTN_DT, _ES_BUFS, _FILL_MODE,
           _DEBUG_OUT, _LOOKAHEAD, _ES8, _KPAD)
    if key not in _NC_CACHE:
        _NC_CACHE[key] = _build_nc(*flags, bench_reps=bench_reps)
    return _NC_CACHE[key]


def _prep_inputs(inputs):
    """Build the 8 per-core input maps from the full problem inputs."""
    x = np.ascontiguousarray(np.asarray(inputs["x"], dtype=np.float32))
    Wq = np.asarray(inputs["Wq"], dtype=np.float32)
    Wk = np.asarray(inputs["Wk"], dtype=np.float32)
    Wv = np.asarray(inputs["Wv"], dtype=np.float32)
    Wo = np.asarray(inputs["Wo"], dtype=np.float32)
    bq = np.asarray(inputs["bq"], dtype=np.float32)
    bk = np.asarray(inputs["bk"], dtype=np.float32)
    bv = np.asarray(inputs["bv"], dtype=np.float32)
    bo = np.asarray(inputs["bo"], dtype=np.float32)
    gamma = np.asarray(inputs["gamma"], dtype=np.float32)
    beta = np.asarray(inputs["beta"], dtype=np.float32)

    has_bias = bool(np.any(bq) or np.any(bk) or np.any(bv))
    has_bo = bool(np.any(bo))
    has_gamma = bool(np.any(gamma != 1.0))
    has_beta = bool(np.any(beta))
    flags = (has_bias, has_bo, has_gamma, has_beta)

    import ml_dtypes
    cast8 = lambda a: np.ascontiguousarray(a, dtype=np.float32).astype(
        ml_dtypes.float8_e4m3)

    def dr_pack(W):
        # [h*128+i, g*512+c] = W[h*256+g*128+i, c]
        out = np.zeros((2 * 128, 2 * D), np.float32)
        for h in range(2):
            for g in range(2):
                r0 = 256 * h + 128 * g
                out[h * 128:(h + 1) * 128, g * D:(g + 1) * D] = W[r0:r0 + 128, :]
        return cast8(out)

    xT = cast8(np.concatenate([x.T, np.ones((1, S), np.float32)], axis=0))
    wq_e = dr_pack(Wq)
    wk_e = dr_pack(Wk)
    wv_e = dr_pack(Wv)
    wo_r = _round_f32r(Wo)

    shared = {
        "xT": xT, "wq": wq_e, "wk": wk_e, "wv": wv_e, "wo": wo_r,
        "ident": np.eye(128, dtype=np.float32),
        "bo": bo.reshape(1, D), "gamma": gamma.reshape(1, D),
        "beta": beta.reshape(1, D),
    }
    if has_bias:
        shared["wqb"] = cast8(bq.reshape(1, D))
        shared["wkb"] = cast8(bk.reshape(1, D))
        shared["wvb"] = cast8(bv.reshape(1, D))
    in_maps = []
    for i in range(NCORES):
        m = dict(shared)
        m["xq"] = np.ascontiguousarray(xT[:, i * SLICE:(i + 1) * SLICE])
        m["x_slice"] = np.ascontiguousarray(x[i * SLICE:(i + 1) * SLICE, :])
        in_maps.append(m)
    return flags, in_maps


def _run(inputs, trace=False, **kw):
    flags, in_maps = _prep_inputs(inputs)
    nc = _get_nc(flags)
    res = run_bass_kernel_spmd(nc, in_maps, core_ids=list(range(NCORES)),
                               trace=trace, **kw)
    out = np.concatenate([res.results[i]["y"] for i in range(NCORES)], axis=0)
    return out, res


def kernel(**inputs) -> np.ndarray:
    out, _ = _run(inputs, trace=False)
    return out



# revision 51
# speedup vs baseline: 1.1703x; 1.1703x over previous
"""Trainium2 Bass kernel for nn_Attention_15109694948028.

Single-layer attention block: QKV proj -> 8-head SDPA (S=4096, d_k=64)
-> out proj -> residual -> LayerNorm.  fp32 I/O.

Sharding: sequence-parallel across 8 NeuronCores.  Core i computes the
full output for query rows [i*512, (i+1)*512) of all 8 heads; K and V
are computed redundantly on every core, so there are NO collectives --
cores are fully independent and the host just concatenates the 8
output slices.  (remote_dma and DRAM collectives are both
non-functional under the axon client runtime, so comm-based K/V
de-duplication is not available.)

On-core dataflow (Q/K/V projections in fp8e4m3 with DoubleRow perf
mode; scores/ctx in bf16; out-proj/transposes in float32r;
residual/LayerNorm in fp32; final rel err ~2.8e-3, budget 2e-2):

  phase A: Q projection via fp8 DoubleRow matmuls (contraction 512 = 2
           DR matmuls of 2x128 packed rows -> 2x fewer PE columns than
           bf16; host packs W as [h*128+i, g*512+c] = W[h*256+g*128+i,c]
           and x^T chunks as [128, 2*N] column-block pairs).  The PSUM
           result is split into 8 per-HEAD zero-padded qt tiles
           (head h's 64 dims at their native partitions, other 64
           partitions memset 0) so scores run full 128-contraction.
  flat attention stream over (round, pair, ktile) steps, k in four
  1024-row rounds (bounds SBUF):
    scores^T st tiles [128 k, 1024 = 2 heads x 512 q] on PE, both
    matmuls share the SAME un-split K pair-tile stationary (the zero
    half of each qt kills the other head's contribution); exp on
    ScalarE (scale=1/8 folds 1/sqrt(d_k); no max-subtract: scores are
    ~N(0,1) so exp cannot overflow).
    FLIPPED ctx matmuls: stationary = es [128 k, 128 q] slice, moving
    = [V_h | ones] [128 k, 65] -> out ctx [128 q, 65] uses all 128
    output partitions; the ones column accumulates the softmax
    denominators.  Per pair the 2x4 qtile regions live in two 1-bank
    PSUM tiles; ONLY the first region matmul of a round carries
    start=True because start resets the whole PSUM bank.
    Emission is software-pipelined ACROSS pair and round boundaries
    with _LOOKAHEAD=2 (scores for step i+2 before ctx for step i) so
    neither ACT nor the in-order PE stalls on the exp round-trip, and
    round r+1's K/V fp8-DR projection matmuls are interleaved as
    filler EMITTED FIRST in each step (no dep on exp -> PE chews them
    while waiting, keeping the HAM clock gate warm).
    Cross-round ctx accumulation lives in SBUF cacc [128 q, 4x65]
    per head (DVE add per round).
  normalize: per (head, qtile) reciprocal of the denominator column
           (DVE) and tensor_scalar_mul into per-pair ctxN tiles laid
           out [128 q, qt x (2 heads' 64 d)] so ONE PE transpose per
           qtile (via f32r identity matmul) yields ctxT pairs
           [128 = 2 heads' d, 512 q].
  phase D: out proj as 4 pair-matmuls per q-subtile (contraction 128)
           + residual (exact fp32 x_slice) + LayerNorm on DVE/ACT --
           INTERLEAVED with the last pair's transposes (q-subtile qs
           starts right after that pair's qt=qs transpose, borrowing
           the idle psB PSUM slots) so the LN chains overlap the
           remaining transposes instead of all queuing at the end.

Benchmarking support: _build_nc(bench_reps=N) wraps the whole body in
an on-device For_i loop (used by test.py to cancel the ~200 ms axon
per-call dispatch latency); the graded kernel() path is loop-free.

Perf history (HW exec, per-rep steady state): baseline bf16 236-270us
-> fp8-DR projections 204us -> padded-128 scores 202us (no change, but
enables shared-stationary scores) -> 2-step lookahead 186us -> filler
first 184us.  Measured-but-rejected: es/vpq in fp8 (196us), manual
tile rings (196us), es bufs 2/3/5/6 (196-258us), SBUF pad shims
(190-222us), LA=3/4 (186-189us).  Known walls: exp stream paces at
~1.4us/step (back-to-back ACT can do ~0.65us but that regime did not
transfer into the kernel; es-buffer-count/placement effects on ACT
write throughput are large, nonlinear, and layout-lottery-like); the
PE-dense floor without exp (ablate noact) is ~176us.
"""

import numpy as np

import concourse.bacc as bacc
import concourse.tile as tile
from concourse import mybir
from concourse.bass_utils import run_bass_kernel_spmd

f32 = mybir.dt.float32
f32r = mybir.dt.float32r
AF = mybir.ActivationFunctionType

S = 4096
D = 512
H = 8
DK = 64
NCORES = 8
SLICE = S // NCORES          # 512 query rows per core
P = 128                      # partitions
HALF = S // 2                # 2048 k rows per half
NKTH = HALF // P             # 16 k-tiles per half
NCHH = HALF // 512           # 4 x^T chunks per half
NQS = SLICE // P             # 4 q-subtiles
KGRP = 2                     # k-tiles per exp batch (2 PSUM banks)
EPS = 1e-5


def _round_f32r(a: np.ndarray) -> np.ndarray:
    """RNE to 11 explicit mantissa bits (matches HW fp32 -> f32r cast)."""
    b = np.ascontiguousarray(a, dtype=np.float32).view(np.uint32)
    lsb = (b >> np.uint32(12)) & np.uint32(1)
    return ((b + np.uint32(0x7FF) + lsb) & np.uint32(0xFFFFF000)).view(np.float32)


import os as _os
_ABLATE = _os.environ.get("KABL", "full")
# attention-path matmul dtype: "f32r" (TF32-class, ~1.6e-5 err),
# "mixed" (f32r projections, bf16 scores/ctx), "bf16" (all bf16, fastest)
_ATTN_DT = "bf16"
_ES_BUFS = int(_os.environ.get("KESBUFS", "4"))
# software-pipeline depth: scores for step i+LOOKAHEAD are emitted before
# ctx for step i, so ACT always has the next exp's input ready and runs
# its fast back-to-back path (~641ns/instr vs ~1450ns isolated)
_LOOKAHEAD = int(_os.environ.get("KLA", "2"))
# es/vpq (ctx operands) in fp8e4m3: halves ACT's es write traffic and the
# PE's ctx stationary reads; exp folds a -2 bias so es stays under fp8 max
# (cancels in the softmax normalization)
_ES8 = _os.environ.get("KES8", "0") == "1"
# SBUF placement shim: a dummy pad tile allocated first shifts every
# subsequent SBUF allocation; engine-vs-engine SBUF contention is strongly
# placement-dependent (measured +-35us swings), so this is a tunable
_KPAD = int(_os.environ.get("KPAD", "0"))
# where next-round K/V projection matmuls are emitted relative to the
# attention stream: "inner" (between ktiles), "pair" (bursts at pair
# boundaries), "round" (all between rounds)
_FILL_MODE = "inner"
_DEBUG_OUT = False
bf16 = mybir.dt.bfloat16
fp8 = mybir.dt.float8e4
DRMODE = mybir.MatmulPerfMode.DoubleRow


def _build_nc(has_bias: bool, has_bo: bool, has_gamma: bool, has_beta: bool,
              bench_reps: int = 0):
    ablate = _ABLATE
    attn_dt = _ATTN_DT
    fill_mode = _FILL_MODE
    # Projections run in fp8e4m3 with DoubleRow perf mode: contraction 512
    # = 2 DR matmuls of 2x128 packed rows, 2x fewer PE columns than bf16.
    proj_dt = fp8
    # dtype of scores matmul operands (qt/ktq tiles)
    sc_dt = f32r if attn_dt == "f32r" else bf16
    # dtype of ctx matmul operands (es/vpq tiles)
    es_dt = fp8 if _ES8 else sc_dt
    nc = bacc.Bacc("TRN2", target_bir_lowering=False, debug=False)

    xT = nc.dram_tensor("xT", [D + 1, S], proj_dt, kind="ExternalInput")
    # DR-packed weights: [h*128+i, g*512+c] = W[h*256+g*128+i, c]
    wq = nc.dram_tensor("wq", [2 * P, 2 * D], proj_dt, kind="ExternalInput")
    wk = nc.dram_tensor("wk", [2 * P, 2 * D], proj_dt, kind="ExternalInput")
    wv = nc.dram_tensor("wv", [2 * P, 2 * D], proj_dt, kind="ExternalInput")
    if has_bias:
        wqb_d = nc.dram_tensor("wqb", [1, D], proj_dt, kind="ExternalInput")
        wkb_d = nc.dram_tensor("wkb", [1, D], proj_dt, kind="ExternalInput")
        wvb_d = nc.dram_tensor("wvb", [1, D], proj_dt, kind="ExternalInput")
    wo = nc.dram_tensor("wo", [D, D], f32r, kind="ExternalInput")
    ident = nc.dram_tensor("ident", [P, P], f32r, kind="ExternalInput")
    xq = nc.dram_tensor("xq", [D + 1, SLICE], proj_dt, kind="ExternalInput")
    xs = nc.dram_tensor("x_slice", [SLICE, D], f32, kind="ExternalInput")
    bo = nc.dram_tensor("bo", [1, D], f32, kind="ExternalInput")
    gamma = nc.dram_tensor("gamma", [1, D], f32, kind="ExternalInput")
    beta = nc.dram_tensor("beta", [1, D], f32, kind="ExternalInput")
    y = nc.dram_tensor("y", [SLICE, D], f32, kind="ExternalOutput")
    if _DEBUG_OUT:
        dbg_cacc = nc.dram_tensor("dbg_cacc", [2 * P, 4 * 65], f32,
                                  kind="ExternalOutput")
        dbg_ctxT = nc.dram_tensor("dbg_ctxT", [P, SLICE], f32,
                                  kind="ExternalOutput")

    with tile.TileContext(nc) as tc:
        def emit_body():
            with (
                tc.tile_pool(name="qt", bufs=1) as qtp,
                tc.tile_pool(name="cacc", bufs=1) as cap,
                tc.tile_pool(name="consts", bufs=1) as cp,
            ):
                if _KPAD:
                    padt = cp.tile([P, _KPAD], f32, tag="pad")
                    nc.gpsimd.memset(padt[:], 0.0)
                eps_t = cp.tile([P, 1], f32, tag="eps")
                nc.gpsimd.memset(eps_t[:], EPS)
                eb_t = None
                if _ES8:
                    eb_t = cp.tile([P, 1], f32, tag="eb")
                    nc.gpsimd.memset(eb_t[:], -2.0)

                def bcast_row(dram_row, tag):
                    r = cp.tile([1, D], f32, tag=f"{tag}_row", name=f"{tag}_row")
                    nc.sync.dma_start(r[:], dram_row)
                    b = cp.tile([P, D], f32, tag=f"{tag}_b", name=f"{tag}_b")
                    nc.gpsimd.partition_broadcast(b[:], r[0:1, :])
                    return b

                bo_b = bcast_row(bo[:], "bo") if has_bo else None
                gamma_b = bcast_row(gamma[:], "gamma") if has_gamma else None
                beta_b = bcast_row(beta[:], "beta") if has_beta else None
                if has_bias:
                    xt_ones = cp.tile([1, S], proj_dt, tag="xt_ones")
                    nc.sync.dma_start(xt_ones[:], xT[D:D + 1, :])

                # per-HEAD query tiles, zero-padded to full 128-partition
                # contraction: qtS[2p] holds h0's Q rows at partitions 0:64
                # (zeros at 64:128), qtS[2p+1] h1's at 64:128 (zeros at
                # 0:64).  Scores then run contraction-128 matmuls against
                # the un-split K pair tile, which streams measurably faster
                # per column than the 64-partition variant.
                qtS = [qtp.tile([P, SLICE], sc_dt, tag=f"qt{hh}", name=f"qt{hh}")
                       for hh in range(H)]
                for pp in range(4):
                    nc.gpsimd.memset(qtS[2 * pp][DK:P, :], 0.0)
                    nc.gpsimd.memset(qtS[2 * pp + 1][0:DK, :], 0.0)
                # flipped-ctx accumulator: [128 q, 4 qtiles x (64 d + denom)]
                caccS = [cap.tile([P, 4 * 65], f32, tag=f"cacc{h}",
                                  name=f"cacc{h}") for h in range(H)]
                ident_t = cp.tile([P, P], f32r, tag="ident")

                # ---- phase A: Q projection from per-core xq ----
                with (
                    tc.tile_pool(name="wqp", bufs=1) as wqp,
                    tc.tile_pool(name="xqp", bufs=1) as xqp,
                    tc.tile_pool(name="psA", bufs=4, space="PSUM") as psA,
                ):
                    wqS = [wqp.tile([P, 2 * D], proj_dt, tag=f"wq{h}",
                                    name=f"wq{h}") for h in range(2)]
                    for h in range(2):
                        nc.sync.dma_start(wqS[h][:], wq[h * P:(h + 1) * P, :])
                    xqS = [xqp.tile([P, 2 * SLICE], proj_dt, tag=f"xq{h}",
                                    name=f"xq{h}") for h in range(2)]
                    for h in range(2):
                        for g in range(2):
                            r0 = 256 * h + 128 * g
                            nc.sync.dma_start(
                                xqS[h][:, g * SLICE:(g + 1) * SLICE],
                                xq[r0:r0 + P, :])
                    if has_bias:
                        wqb = wqp.tile([1, D], proj_dt, tag="wqb")
                        nc.sync.dma_start(wqb[:], wqb_d[:])
                        xq_ones = wqp.tile([1, SLICE], proj_dt, tag="xq_ones")
                        nc.sync.dma_start(xq_ones[:], xq[D:D + 1, :])
                    wqv = [t.rearrange("p (g c) -> p g c", g=2) for t in wqS]
                    xqv = [t.rearrange("p (g c) -> p g c", g=2) for t in xqS]
                    NQ4 = SLICE // 4
                    for p in range(4):
                        ps = psA.tile([P, SLICE], f32, tag="psA")
                        idx = 0
                        for n in range(2):
                            for h in range(2):
                                nc.tensor.matmul(
                                    ps[:, n * NQ4 * 2:(n + 1) * NQ4 * 2],
                                    wqv[h][:, :, p * P:(p + 1) * P],
                                    xqv[h][:, :, n * NQ4 * 2:(n + 1) * NQ4 * 2],
                                    start=(idx == 0),
                                    stop=(idx == 3 and not has_bias),
                                    perf_mode=DRMODE, skip_group_check=True,
                                )
                                idx += 1
                        if has_bias:
                            nc.tensor.matmul(
                                ps[:], wqb[0:1, p * P:(p + 1) * P], xq_ones[:],
                                start=False, stop=True,
                            )
                        nc.vector.tensor_copy(qtS[2 * p][0:DK, :], ps[0:DK, :])
                        nc.vector.tensor_copy(qtS[2 * p + 1][DK:P, :],
                                              ps[DK:P, :])

                # ---- round loop: K/V proj per k-quarter + attention ----
                # Round r+1's K/V projection work is EMITTED between round
                # r's attention pairs: engines execute their streams in
                # emission order, so this is what actually interleaves the
                # projection matmuls into the ACT(exp)-paced attention.
                QK = S // 4          # 1024 k rows per round
                NKTQ = QK // P       # 8 k-tiles per round
                with (
                    tc.tile_pool(name="wkv", bufs=1) as wkvp,
                    tc.tile_pool(name="ktq", bufs=2) as ktqp,
                    tc.tile_pool(name="vpq", bufs=2) as vpqp,
                    tc.tile_pool(name="xtc", bufs=2) as xtcp,
                    tc.tile_pool(name="es", bufs=_ES_BUFS) as esp,
                    tc.tile_pool(name="nrm", bufs=1) as nrmp,
                    tc.tile_pool(name="ctxT", bufs=1) as ctp,
                ):
                    wkS = [wkvp.tile([P, 2 * D], proj_dt, tag=f"wk{h}",
                                     name=f"wk{h}") for h in range(2)]
                    wvS = [wkvp.tile([P, 2 * D], proj_dt, tag=f"wv{h}",
                                     name=f"wv{h}") for h in range(2)]
                    for h in range(2):
                        nc.sync.dma_start(wkS[h][:], wk[h * P:(h + 1) * P, :])
                    wkv = [t.rearrange("p (g c) -> p g c", g=2) for t in wkS]
                    wvv = [t.rearrange("p (g c) -> p g c", g=2) for t in wvS]
                    if has_bias:
                        wkb = wkvp.tile([1, D], proj_dt, tag="wkb")
                        wvb = wkvp.tile([1, D], proj_dt, tag="wvb")
                        nc.sync.dma_start(wkb[:], wkb_d[:])
                        nc.sync.dma_start(wvb[:], wvb_d[:])
                    # per-pair normalized ctx [128 q, 4 qtiles x (2 heads' d)]
                    # and per-pair transposed ctx [128 = 2 heads' d, 512 q]
                    ctxN = [ctp.tile([P, 4 * 2 * DK], f32r, tag=f"ctxN{pp}",
                                     name=f"ctxN{pp}") for pp in range(4)]
                    ctxTp = [ctp.tile([P, SLICE], f32r, tag=f"ctxTp{pp}",
                                      name=f"ctxTp{pp}") for pp in range(4)]
                    woS = [wkvp.tile([P, D], f32r, tag=f"wo{pp}",
                                     name=f"wo{pp}") for pp in range(4)]

                    with (
                        tc.tile_pool(name="st", bufs=2, space="PSUM") as stp,
                        tc.tile_pool(name="psB", bufs=2, space="PSUM") as psBp,
                        tc.tile_pool(name="ctxps", bufs=2, space="PSUM") as cpp,
                        tc.tile_pool(name="ln", bufs=2) as lnp,
                    ):
                        ktq_of, vpq_of, xc_of = {}, {}, {}

                        def alloc_round(r):
                            ktq_of[r] = [ktqp.tile([P, QK], sc_dt, tag=f"ktq{p}",
                                                   name=f"ktq{p}") for p in range(4)]
                            vpq_of[r] = [vpqp.tile([P, H * 65], es_dt, tag=f"vpq{t}",
                                                   name=f"vpq{t}") for t in range(NKTQ)]
                            xc_of[r] = [xtcp.tile([P, 2 * QK], proj_dt, tag=f"xtc{h}",
                                                  name=f"xtc{h}") for h in range(2)]
                            k0 = r * QK
                            for h in range(2):
                                for g in range(2):
                                    r0 = 256 * h + 128 * g
                                    nc.sync.dma_start(
                                        xc_of[r][h][:, g * QK:(g + 1) * QK],
                                        xT[r0:r0 + P, k0:k0 + QK])

                        def kv_fill_gen(r):
                            """Yield after each PE op of round r's K/V
                            projection -- consumed one step at a time as
                            filler inside the previous round's attention."""
                            if ablate == "noproj":
                                for p in range(4):
                                    kv = ktq_of[r][p][:]
                                    nc.gpsimd.memset(kv.bitcast(f32) if sc_dt == f32r else kv, 0.5)
                                for t in range(NKTQ):
                                    vv = vpq_of[r][t][:]
                                    nc.gpsimd.memset(vv.bitcast(f32) if es_dt == f32r else vv, 0.5)
                                return
                            k0 = r * QK
                            xc, ktq, vpq = xc_of[r], ktq_of[r], vpq_of[r]
                            xcv = [t.rearrange("p (g c) -> p g c", g=2)
                                   for t in xc]
                            for cc in range(QK // 512):
                                for p in range(4):
                                    ps = psBp.tile([P, 512], f32, tag="psB")
                                    idx = 0
                                    for n in range(2):
                                        for h in range(2):
                                            nc.tensor.matmul(
                                                ps[:, n * 256:(n + 1) * 256],
                                                wkv[h][:, :, p * P:(p + 1) * P],
                                                xcv[h][:, :, cc * 512 + n * 256:
                                                       cc * 512 + (n + 1) * 256],
                                                start=(idx == 0),
                                                stop=(idx == 3 and not has_bias),
                                                perf_mode=DRMODE,
                                                skip_group_check=True,
                                            )
                                            idx += 1
                                            if idx == 2:
                                                yield
                                    if has_bias:
                                        nc.tensor.matmul(
                                            ps[:], wkb[0:1, p * P:(p + 1) * P],
                                            xt_ones[0:1, k0 + cc * 512:k0 + (cc + 1) * 512],
                                            start=False, stop=True,
                                        )
                                    nc.vector.tensor_copy(
                                        ktq[p][:, cc * 512:(cc + 1) * 512], ps[:])
                                    yield
                            for lt in range(NKTQ):
                                ps = psBp.tile([P, D], f32, tag="psB")
                                idx = 0
                                for n in range(2):
                                    for h in range(2):
                                        nc.tensor.matmul(
                                            ps[:, n * 256:(n + 1) * 256],
                                            xcv[h][:, :, lt * P:(lt + 1) * P],
                                            wvv[h][:, :, n * 256:(n + 1) * 256],
                                            start=(idx == 0),
                                            stop=(idx == 3 and not has_bias),
                                            perf_mode=DRMODE,
                                            skip_group_check=True,
                                        )
                                        idx += 1
                                        if idx == 2:
                                            yield
                                if has_bias:
                                    nc.tensor.matmul(
                                        ps[:],
                                        xt_ones[0:1, k0 + lt * P:k0 + (lt + 1) * P],
                                        wvb[:], start=False, stop=True,
                                    )
                                vt = vpq[lt]
                                v3 = vt.rearrange("p (h c) -> p h c", c=65)
                                # ones columns on DVE (not Pool): the ctx
                                # matmul already waits on the DVE V-copy, so
                                # this adds no cross-engine hop
                                if es_dt == f32r:
                                    nc.vector.memset(v3[:, :, 64:65].bitcast(f32), 1.0)
                                else:
                                    nc.vector.memset(v3[:, :, 64:65], 1.0)
                                nc.vector.tensor_copy(
                                    v3[:, :, 0:64],
                                    ps[:].rearrange("p (h d) -> p h d", d=DK))
                                yield

                        def normalize(h):
                            # denom col qt*65+64 -> rc[:, qt]; ctxN = cacc * rc
                            # head h's d-block sits at cols qt*128 + (h%2)*64
                            # of the pair tile so one transpose per qtile
                            # yields both heads stacked on partitions
                            rc = nrmp.tile([P, 4], f32, tag="rc", bufs=2)
                            cN = ctxN[h // 2]
                            off = (h % 2) * DK
                            for qt in range(4):
                                nc.vector.reciprocal(
                                    rc[:, qt:qt + 1],
                                    caccS[h][:, qt * 65 + 64:qt * 65 + 65])
                            for qt in range(4):
                                nc.vector.tensor_scalar_mul(
                                    cN[:, qt * P + off:qt * P + off + DK],
                                    caccS[h][:, qt * 65:qt * 65 + DK],
                                    rc[:, qt:qt + 1])

                        def emit_phase_d(qs):
                            # out proj + residual + LayerNorm for q-subtile
                            # qs, PSUM slot borrowed from the idle psB pool
                            op = psBp.tile([P, D], f32, tag="psB",
                                           name=f"psD{qs}")
                            for pp in range(4):
                                nc.tensor.matmul(
                                    op[:], ctxTp[pp][:, qs * P:(qs + 1) * P],
                                    woS[pp][:],
                                    start=(pp == 0), stop=(pp == 3),
                                )
                            xt_ = lnp.tile([P, D], f32, tag="xres")
                            nc.sync.dma_start(xt_[:], xs[qs * P:(qs + 1) * P, :])
                            t = lnp.tile([P, D], f32, tag="t")
                            nc.vector.tensor_add(t[:], op[:], xt_[:])
                            if has_bo:
                                nc.vector.tensor_add(t[:], t[:], bo_b[:])
                            s1 = lnp.tile([P, 1], f32, tag="s1")
                            nc.vector.reduce_sum(s1[:], t[:],
                                                 axis=mybir.AxisListType.X)
                            negmu = lnp.tile([P, 1], f32, tag="negmu")
                            nc.vector.tensor_scalar_mul(negmu[:], s1[:], -1.0 / D)
                            tcen = lnp.tile([P, D], f32, tag="tcen")
                            nc.vector.tensor_scalar_add(tcen[:], t[:], negmu[:])
                            sq = lnp.tile([P, D], f32, tag="sq")
                            v1 = lnp.tile([P, 1], f32, tag="v1")
                            nc.scalar.activation(sq[:], tcen[:], AF.Square,
                                                 accum_out=v1[:])
                            std = lnp.tile([P, 1], f32, tag="std")
                            nc.scalar.activation(std[:], v1[:], AF.Sqrt,
                                                 bias=eps_t[:], scale=1.0 / D)
                            rstd = lnp.tile([P, 1], f32, tag="rstd")
                            nc.vector.reciprocal(rstd[:], std[:])
                            out_t = lnp.tile([P, D], f32, tag="out_t")
                            nc.vector.tensor_scalar_mul(out_t[:], tcen[:], rstd[:])
                            if has_gamma:
                                nc.vector.tensor_mul(out_t[:], out_t[:], gamma_b[:])
                            if has_beta:
                                nc.vector.tensor_add(out_t[:], out_t[:], beta_b[:])
                            nc.sync.dma_start(y[qs * P:(qs + 1) * P, :], out_t[:])

                        def transpose_pair(p):
                            # ctxN [128 q, 128 = 2 heads' d] blocks ->
                            # ctxTp [128 d, 512 q] via one PE transpose each.
                            # For the LAST pair, out-proj + LN for q-subtile
                            # qt follows immediately after qt's transpose so
                            # the phase-D chains overlap the remaining
                            # transposes instead of all waiting at the end.
                            if ablate == "notrans":
                                nc.vector.tensor_copy(ctxTp[p][:], qtS[p][:])
                                if p == 3:
                                    for qs in range(NQS):
                                        emit_phase_d(qs)
                                return
                            for qt in range(4):
                                psT = psBp.tile([P, P], f32r, tag="psB")
                                nc.tensor.matmul(
                                    psT[:],
                                    ctxN[p][:, qt * P:(qt + 1) * P],
                                    ident_t[:], is_transpose=True,
                                )
                                nc.vector.tensor_copy(
                                    ctxTp[p][:, qt * P:(qt + 1) * P], psT[:])
                                if p == 3:
                                    emit_phase_d(qt)

                        esq = []
                        ctx_of = {}

                        es_const = None
                        if ablate in ("desync", "noact"):
                            es_const = ctp.tile([P, 1024], es_dt,
                                                tag="es_const")
                            nc.gpsimd.memset(es_const[:], 0.5)
                        sttz = None
                        if ablate == "noscore":
                            sttz = stp.tile([P, 1024], f32, tag="st")
                            nc.vector.memset(sttz[:], 0.25)

                        def emit_s(rnd, p, lkt):
                            ktq = ktq_of[rnd]
                            si = (rnd * 4 + p) * NKTQ + lkt
                            if ablate != "noscore":
                                stt = stp.tile([P, 1024], f32, tag="st")
                                nc.tensor.matmul(
                                    stt[:, 0:512],
                                    ktq[p][:, lkt * P:(lkt + 1) * P],
                                    qtS[2 * p][:],
                                    start=True, stop=True,
                                )
                                nc.tensor.matmul(
                                    stt[:, 512:1024],
                                    ktq[p][:, lkt * P:(lkt + 1) * P],
                                    qtS[2 * p + 1][:],
                                    start=True, stop=True,
                                )
                            if ablate == "noact":
                                esq.append(es_const)
                                return
                            es = esp.tile([P, 1024], es_dt, tag="es")
                            kw = dict(bias=eb_t[:]) if _ES8 else {}
                            if ablate == "dveexp":
                                nc.vector.tensor_copy(es[:], stt[:])
                            elif ablate == "noscore":
                                nc.scalar.activation(es[:], sttz[:], AF.Exp,
                                                     scale=0.125, **kw)
                            else:
                                nc.scalar.activation(es[:], stt[:], AF.Exp,
                                                     scale=0.125, **kw)
                            esq.append(es_const if ablate == "desync" else es)

                        def emit_c(rnd, p, lkt):
                            # flipped ctx: stationary = es [128 k, 128 q],
                            # moving = [V_h | ones] [128 k, 65]; out
                            # [128 q, 65] regions; alternate the two
                            # PSUM banks (ctx0/ctx1) between matmuls
                            vpq = vpq_of[rnd]
                            h0, h1 = 2 * p, 2 * p + 1
                            es = esq.pop(0)
                            if lkt == 0:
                                cA = cpp.tile([P, 4 * 65], f32, tag="ctxu",
                                              name=f"ctxu{rnd}_{p}a")
                                cB = cpp.tile([P, 4 * 65], f32, tag="ctxu",
                                              name=f"ctxu{rnd}_{p}b")
                                ctx_of[p] = (cA, cB)
                            ctx0, ctx1 = ctx_of[p]
                            if ablate == "noctx":
                                if lkt == 0:
                                    nc.tensor.matmul(
                                        ctx0[:, 0:260], es[:, 0:128],
                                        vpq[lkt][:, 0:260],
                                        start=True, stop=True)
                                    nc.tensor.matmul(
                                        ctx1[:, 0:260], es[:, 0:128],
                                        vpq[lkt][:, 0:260],
                                        start=True, stop=True)
                                return
                            for qt in range(4):
                                for hb, ctx, hh in ((0, ctx0, h0),
                                                    (1, ctx1, h1)):
                                    # start=True resets the WHOLE PSUM
                                    # bank, so only the first region
                                    # matmul of the round may carry it
                                    nc.tensor.matmul(
                                        ctx[:, qt * 65:(qt + 1) * 65],
                                        es[:, hb * 512 + qt * P:
                                           hb * 512 + (qt + 1) * P],
                                        vpq[lkt][:, hh * 65:(hh + 1) * 65],
                                        start=(lkt == 0 and qt == 0),
                                        stop=(lkt == NKTQ - 1 and qt == 3),
                                        skip_group_check=True,
                                    )

                        def finish_pair(rnd, p):
                            h0, h1 = 2 * p, 2 * p + 1
                            ctx0, ctx1 = ctx_of[p]
                            for h, ctxu in ((h0, ctx0), (h1, ctx1)):
                                if rnd == 0:
                                    nc.vector.tensor_copy(caccS[h][:], ctxu[:])
                                else:
                                    nc.vector.tensor_add(caccS[h][:], ctxu[:],
                                                         caccS[h][:])
                                if rnd == 3:
                                    normalize(h)
                            if rnd == 3:
                                transpose_pair(p)

                        alloc_round(0)
                        # wv/wo/ident are needed later than round 0's first
                        # K-proj matmuls: issue them after the x^T chunks so
                        # the fill isn't stuck behind cold weights
                        for h in range(2):
                            nc.sync.dma_start(wvS[h][:], wv[h * P:(h + 1) * P, :])
                        for pp in range(4):
                            nc.sync.dma_start(woS[pp][:], wo[pp * P:(pp + 1) * P, :])
                        nc.sync.dma_start(ident_t[:], ident[:])
                        for _ in kv_fill_gen(0):
                            pass
                        if ablate != "noattn":
                            # flat software-pipelined stream over all
                            # (round, pair, ktile) steps: scores for step
                            # i+1 are emitted before ctx for step i, ACROSS
                            # pair and round boundaries, so the in-order PE
                            # never stalls on exp at pair starts
                            steps = [(r, p, k) for r in range(4)
                                     for p in range(4) for k in range(NKTQ)]
                            filler = None
                            la = _LOOKAHEAD
                            for j in range(la):
                                emit_s(*steps[j])
                            alloc_round(1)
                            for i, (rnd, p, lkt) in enumerate(steps):
                                if lkt == 0 and p == 0:
                                    if filler is not None:
                                        for _ in filler:
                                            pass
                                    filler = (kv_fill_gen(rnd + 1)
                                              if rnd < 3 else None)
                                if lkt == 0 and p == 2 and rnd < 2:
                                    # prefetch round rnd+2's x^T chunks a
                                    # full round before its fillers run
                                    alloc_round(rnd + 2)
                                # filler FIRST: the proj matmuls have no dep
                                # on exp, so the in-order PE chews them while
                                # waiting for exp to free the stt buffer /
                                # produce es -- no micro-idles (HAM stays
                                # warm at 2.4GHz)
                                if filler is not None:
                                    next(filler, None)
                                if i + la < len(steps):
                                    emit_s(*steps[i + la])
                                emit_c(rnd, p, lkt)
                                if lkt == NKTQ - 1:
                                    finish_pair(rnd, p)
                            if filler is not None:
                                for _ in filler:
                                    pass
                        else:
                            for rnd in range(4):
                                if rnd < 3:
                                    alloc_round(rnd + 1)
                                    for _ in kv_fill_gen(rnd + 1):
                                        pass
                        if ablate == "noattn":
                            for h in range(H):
                                nc.gpsimd.memset(caccS[h][:], 1.0)
                                normalize(h)
                            for p in range(4):
                                transpose_pair(p)


        if bench_reps:
            with tc.For_i(0, bench_reps, 1):
                emit_body()
        else:
            emit_body()
    nc.compile()
    return nc


_NC_CACHE: dict = {}


def _get_nc(flags, bench_reps: int = 0):
    key = (flags, bench_reps, _ABLATE, _ATTN_DT, _ES_BUFS, _FILL_MODE,
           _DEBUG_OUT, _LOOKAHEAD, _ES8, _KPAD)
    if key not in _NC_CACHE:
        _NC_CACHE[key] = _build_nc(*flags, bench_reps=bench_reps)
    return _NC_CACHE[key]


def _prep_inputs(inputs):
    """Build the 8 per-core input maps from the full problem inputs."""
    x = np.ascontiguousarray(np.asarray(inputs["x"], dtype=np.float32))
    Wq = np.asarray(inputs["Wq"], dtype=np.float32)
    Wk = np.asarray(inputs["Wk"], dtype=np.float32)
    Wv = np.asarray(inputs["Wv"], dtype=np.float32)
    Wo = np.asarray(inputs["Wo"], dtype=np.float32)
    bq = np.asarray(inputs["bq"], dtype=np.float32)
    bk = np.asarray(inputs["bk"], dtype=np.float32)
    bv = np.asarray(inputs["bv"], dtype=np.float32)
    bo = np.asarray(inputs["bo"], dtype=np.float32)
    gamma = np.asarray(inputs["gamma"], dtype=np.float32)
    beta = np.asarray(inputs["beta"], dtype=np.float32)

    has_bias = bool(np.any(bq) or np.any(bk) or np.any(bv))
    has_bo = bool(np.any(bo))
    has_gamma = bool(np.any(gamma != 1.0))
    has_beta = bool(np.any(beta))
    flags = (has_bias, has_bo, has_gamma, has_beta)

    import ml_dtypes
    cast8 = lambda a: np.ascontiguousarray(a, dtype=np.float32).astype(
        ml_dtypes.float8_e4m3)

    def dr_pack(W):
        # [h*128+i, g*512+c] = W[h*256+g*128+i, c]
        out = np.zeros((2 * 128, 2 * D), np.float32)
        for h in range(2):
            for g in range(2):
                r0 = 256 * h + 128 * g
                out[h * 128:(h + 1) * 128, g * D:(g + 1) * D] = W[r0:r0 + 128, :]
        return cast8(out)

    xT = cast8(np.concatenate([x.T, np.ones((1, S), np.float32)], axis=0))
    wq_e = dr_pack(Wq)
    wk_e = dr_pack(Wk)
    wv_e = dr_pack(Wv)
    wo_r = _round_f32r(Wo)

    shared = {
        "xT": xT, "wq": wq_e, "wk": wk_e, "wv": wv_e, "wo": wo_r,
        "ident": np.eye(128, dtype=np.float32),
        "bo": bo.reshape(1, D), "gamma": gamma.reshape(1, D),
        "beta": beta.reshape(1, D),
    }
    if has_bias:
        shared["wqb"] = cast8(bq.reshape(1, D))
        shared["wkb"] = cast8(bk.reshape(1, D))
        shared["wvb"] = cast8(bv.reshape(1, D))
    in_maps = []
    for i in range(NCORES):
        m = dict(shared)
        m["xq"] = np.ascontiguousarray(xT[:, i * SLICE:(i + 1) * SLICE])
        m["x_slice"] = np.ascontiguousarray(x[i * SLICE:(i + 1) * SLICE, :])
        in_maps.append(m)
    return flags, in_maps


def _run(inputs, trace=False, **kw):
    flags, in_maps = _prep_inputs(inputs)
    nc = _get_nc(flags)
    res = run_bass_kernel_spmd(nc, in_maps, core_ids=list(range(NCORES)),
                               trace=trace, **kw)
    out = np.concatenate([res.results[i]["y"] for i in range(NCORES)], axis=0)
    return out, res


def kernel(**inputs) -> np.ndarray:
    out, _ = _run(inputs, trace=False)
    return out

